# revision 4
# baseline (speedup 1.0000x reference)
"""GraphTransformer (4-layer GAT) on 8 TRN2 NeuronCores.

Strategy (dst-sharded graph parallel):
  - Core k owns dst nodes [k*6250, (k+1)*6250). Edges (incl. self loops)
    are grouped by dst block (128 dst nodes per block), sorted on host.
  - Per layer: each core computes hx rows for its own nodes (x_own @ W),
    all-gathers the bf16 hx table [N,128] to every core's DRAM, then
    edge-gathers hx[src] rows with gpsimd.dma_gather (256B rows).
  - Segment softmax without max-subtraction (scores are O(1), exp is safe):
    alpha = exp(leaky(a_s+a_d)) / segsum. a_s computed on-chip from the
    gathered rows (DVE dot); a_d expanded per-edge via Mt-tile matmul.
  - Aggregation = matmul with host-built one-hot M tiles:
    psum[128 dst, fo+H] += M_tile.T @ (w * hx_rows | w), accumulating
    numerator and denominator in one PE pass per edge tile.
  - int16 gather indices limit -> the hx table is gathered in two halves
    (src < N/2 and src >= N/2); per (block, half) edge lists are padded to
    128-edge tiles with ghost edges (src=0, zero M row).
"""

import sys

sys.path.insert(0, "/opt/trn_rl_repo")

import numpy as np
import ml_dtypes

from concourse import bass, mybir, bacc, tile
from concourse.tile import TileContext
from concourse.bass_utils import run_bass_kernel_spmd
from concourse import library_config
from concourse.masks import make_identity
from concourse._compat import cdiv, get_trn_type

BF16 = mybir.dt.bfloat16
F32 = mybir.dt.float32
I16 = mybir.dt.int16
AF = mybir.ActivationFunctionType
ALU = mybir.AluOpType

IN_CH, HID = 74, 32
HEADS = [4, 4, 4, 1]
NEG_SLOPE = 0.2
BN_EPS = 1e-5
N_CORES = 8

NP_BF16 = ml_dtypes.bfloat16


# ----------------------------------------------------------------------------
# host-side graph prep
# ----------------------------------------------------------------------------

def _wrap_idx(arr):
    """[n] int array (n % 16 == 0) -> [128, n//16] int16 wrapped+replicated."""
    a16 = np.asarray(arr, np.int16).reshape(-1, 16).T.copy()
    return np.tile(a16, (8, 1))


def prep_graph(edge_index, n_nodes, n_cores):
    """Partition + sort edges, build per-core gather indices and M/Mt tiles."""
    shard = n_nodes // n_cores
    nblk = cdiv(shard, 128)
    shard_pad = nblk * 128
    half = n_nodes // 2

    src = np.asarray(edge_index[0], np.int64)
    dst = np.asarray(edge_index[1], np.int64)
    loops = np.arange(n_nodes, dtype=np.int64)
    src = np.concatenate([src, loops])
    dst = np.concatenate([dst, loops])
    core = dst // shard
    dl = dst % shard

    # per core / per block / per half edge lists
    lists = []  # lists[k][b] = ((lo_src, lo_dcol), (hi_src, hi_dcol))
    for k in range(n_cores):
        m = core == k
        sk, dlk = src[m], dl[m]
        if shard_pad > shard:  # ghost self-ish edges for pad dst slots
            gp = np.arange(shard, shard_pad)
            sk = np.concatenate([sk, np.zeros(len(gp), np.int64)])
            dlk = np.concatenate([dlk, gp])
        bidx = dlk // 128
        blocks = []
        for b in range(nblk):
            mm = bidx == b
            s_b, d_b = sk[mm], dlk[mm] % 128
            lo = s_b < half
            blocks.append(
                ((s_b[lo], d_b[lo]), (s_b[~lo] - half, d_b[~lo]))
            )
        lists.append(blocks)

    TLO = [
        max(cdiv(max(len(lists[k][b][0][0]), 1), 128) for k in range(n_cores))
        for b in range(nblk)
    ]
    THI = [
        max(cdiv(max(len(lists[k][b][1][0]), 1), 128) for k in range(n_cores))
        for b in range(nblk)
    ]
    t_tot = sum(TLO) + sum(THI)

    per_core = []
    for k in range(n_cores):
        idx_lo = np.zeros(sum(TLO) * 128, np.int64)
        idx_hi = np.zeros(sum(THI) * 128, np.int64)
        m_t = np.zeros((128, t_tot, 128), NP_BF16)
        mt_t = np.zeros((128, t_tot, 128), NP_BF16)
        olo = ohi = toff = 0
        for b in range(nblk):
            (ls, ld), (hs, hd) = lists[k][b]
            idx_lo[olo : olo + len(ls)] = ls
            idx_hi[ohi : ohi + len(hs)] = hs
            # M tiles for this block: lo tiles then hi tiles
            for src_arr, dcol_arr, base in (
                (ls, ld, toff),
                (hs, hd, toff + TLO[b]),
            ):
                j = np.arange(len(src_arr))
                p, t = j % 128, j // 128
                m_t[p, base + t, dcol_arr] = 1.0
                mt_t[dcol_arr, base + t, p] = 1.0
            olo += TLO[b] * 128
            ohi += THI[b] * 128
            toff += TLO[b] + THI[b]
        per_core.append(
            dict(
                idx_lo=_wrap_idx(idx_lo),
                idx_hi=_wrap_idx(idx_hi),
                m_tiles=m_t.reshape(128, t_tot * 128),
                mt_tiles=mt_t.reshape(128, t_tot * 128),
            )
        )
    meta = dict(
        shard=shard, nblk=nblk, shard_pad=shard_pad, half=half,
        TLO=TLO, THI=THI, t_tot=t_tot,
    )
    return per_core, meta


def prep_weights(params):
    """Per-layer device constants: Wcat=[W | W@a_dst_flat], a_src bcast,
    fused BN scale/shift bcast."""
    out = []
    for p in params:
        W = np.asarray(p["W"], np.float32)          # [fi, fo]
        a_src = np.asarray(p["a_src"], np.float32)  # [H, C]
        a_dst = np.asarray(p["a_dst"], np.float32)
        bias = np.asarray(p["bias"], np.float32)
        gamma = np.asarray(p["gamma"], np.float32)
        beta = np.asarray(p["beta"], np.float32)
        rm = np.asarray(p["rm"], np.float32)
        rv = np.asarray(p["rv"], np.float32)
        h, c = a_src.shape
        fi, fo = W.shape
        # a_d[n,h] = sum_c (x@W)[n,h,c] * a_dst[h,c]  ->  x @ Wd
        Wd = (W.reshape(fi, h, c) * a_dst[None]).sum(-1)  # [fi, h]
        wcat = np.concatenate([W, Wd], axis=1)       # [fi, fo+h]
        asrc_flat = a_src.reshape(fo)
        scale = gamma / np.sqrt(rv + BN_EPS)
        shift = beta + scale * (bias - rm)
        out.append(
            dict(
                wcat=wcat,
                asrc=np.tile(asrc_flat[None, :], (128, 1)).astype(np.float32),
                scale=np.tile(scale[None, :], (128, 1)).astype(np.float32),
                shift=np.tile(shift[None, :], (128, 1)).astype(np.float32),
                fi=fi, fo=fo, h=h,
            )
        )
    return out


# ----------------------------------------------------------------------------
# device kernel
# ----------------------------------------------------------------------------

def build_nc(meta, wmeta, n_nodes, n_cores, edge_bufs=3):
    shard, nblk, shard_pad = meta["shard"], meta["nblk"], meta["shard_pad"]
    half = meta["half"]
    TLO, THI, t_tot = meta["TLO"], meta["THI"], meta["t_tot"]
    n_layers = len(wmeta)
    t_max = max(TLO[b] + THI[b] for b in range(nblk))
    hmax = max(w["h"] for w in wmeta)

    nc = bacc.Bacc(get_trn_type() or "TRN2", target_bir_lowering=False)

    fi0 = wmeta[0]["fi"]
    x0 = nc.dram_tensor("xt0", [fi0, shard_pad], F32, kind="ExternalInput")
    idx_lo_d = nc.dram_tensor("idx_lo", [128, sum(TLO) * 8], I16,
                              kind="ExternalInput")
    idx_hi_d = nc.dram_tensor("idx_hi", [128, sum(THI) * 8], I16,
                              kind="ExternalInput")
    m_d = nc.dram_tensor("m_tiles", [128, t_tot * 128], BF16,
                         kind="ExternalInput")
    mt_d = nc.dram_tensor("mt_tiles", [128, t_tot * 128], BF16,
                          kind="ExternalInput")
    wc_d, as_d, sc_d, sh_d = [], [], [], []
    for l, w in enumerate(wmeta):
        wc_d.append(nc.dram_tensor(f"wcat{l}", [w["fi"], w["fo"] + w["h"]],
                                   F32, kind="ExternalInput"))
        as_d.append(nc.dram_tensor(f"asrc{l}", [128, w["fo"]], F32,
                                   kind="ExternalInput"))
        sc_d.append(nc.dram_tensor(f"scale{l}", [128, w["fo"]], F32,
                                   kind="ExternalInput"))
        sh_d.append(nc.dram_tensor(f"shift{l}", [128, w["fo"]], F32,
                                   kind="ExternalInput"))
    out_d = nc.dram_tensor("out", [shard, wmeta[-1]["fo"]], F32,
                           kind="ExternalOutput")
    ag_in = [nc.dram_tensor(f"ag_in{l}", [shard, 128], BF16)
             for l in range(n_layers)]
    table = [nc.dram_tensor(f"table{l}", [n_nodes, 128], BF16,
                            addr_space="Shared") for l in range(n_layers)]

    rg = [list(range(n_cores))]

    with TileContext(nc) as tc:
        with (
            tc.tile_pool(name="const", bufs=1) as constp,
            tc.tile_pool(name="store", bufs=1) as storep,
            tc.tile_pool(name="edge", bufs=edge_bufs) as edgep,
            tc.tile_pool(name="mm", bufs=edge_bufs) as mp,
            tc.tile_pool(name="mtm", bufs=edge_bufs) as mtp,
            tc.tile_pool(name="gg", bufs=edge_bufs) as gp,
            tc.tile_pool(name="sc", bufs=4) as scp,
            tc.tile_pool(name="hx", bufs=3) as hxp,
            tc.tile_pool(name="ppro", bufs=2, space="PSUM") as ppro,
            tc.tile_pool(name="pad", bufs=2, space="PSUM") as padp,
            tc.tile_pool(name="pblk", bufs=2, space="PSUM") as pblkp,
            tc.tile_pool(name="pt", bufs=2, space="PSUM") as ptp,
        ):
            nc.gpsimd.load_library(library_config.mlp)

            ident = constp.tile([128, 128], F32, tag="ident")
            make_identity(nc, ident[:])

            idx_lo = constp.tile([128, sum(TLO) * 8], I16, tag="ixlo")
            idx_hi = constp.tile([128, sum(THI) * 8], I16, tag="ixhi")
            nc.sync.dma_start(idx_lo[:], idx_lo_d[:])
            nc.sync.dma_start(idx_hi[:], idx_hi_d[:])

            wcs, ass, scs, shs = [], [], [], []
            for l, w in enumerate(wmeta):
                t = constp.tile([w["fi"], w["fo"] + w["h"]], F32, tag=f"wc{l}")
                nc.sync.dma_start(t[:], wc_d[l][:])
                wcs.append(t)
                t = constp.tile([128, w["fo"]], F32, tag=f"as{l}")
                nc.sync.dma_start(t[:], as_d[l][:])
                ass.append(t)
                t = constp.tile([128, w["fo"]], F32, tag=f"sc{l}")
                nc.sync.dma_start(t[:], sc_d[l][:])
                scs.append(t)
                t = constp.tile([128, w["fo"]], F32, tag=f"sh{l}")
                nc.sync.dma_start(t[:], sh_d[l][:])
                shs.append(t)

            xt_a = storep.tile([128, shard_pad], F32, tag="xt_a")
            xt_b = storep.tile([128, shard_pad], F32, tag="xt_b")
            ad_store = storep.tile([128, nblk, hmax], BF16, tag="ad")
            nc.sync.dma_start(xt_a[:fi0, :], x0[:])

            for l, w in enumerate(wmeta):
                fi, fo, H = w["fi"], w["fo"], w["h"]
                xt_cur = xt_a if l % 2 == 0 else xt_b
                xt_next = xt_b if l % 2 == 0 else xt_a

                # ---- prologue: hx rows + a_d for own nodes ----
                for b in range(nblk):
                    pp = ppro.tile([128, fo + H], F32, tag="pro")
                    nc.tensor.matmul(
                        pp[:], lhsT=xt_cur[:fi, b * 128 : (b + 1) * 128],
                        rhs=wcs[l][:], start=True, stop=True,
                    )
                    hxb = hxp.tile([128, fo], BF16, tag="hxb")
                    nc.scalar.copy(hxb[:], pp[:, :fo])
                    nc.vector.tensor_copy(ad_store[:, b, :H], pp[:, fo:])
                    nrows = min(128, shard - b * 128)
                    nc.sync.dma_start(
                        ag_in[l][b * 128 : b * 128 + nrows, :fo],
                        hxb[:nrows, :],
                    )

                nc.gpsimd.collective_compute(
                    "AllGather", ALU.bypass, replica_groups=rg,
                    ins=[ag_in[l][:]], outs=[table[l][:]],
                )

                # ---- edge phase ----
                olo = ohi = toff = 0
                for b in range(nblk):
                    TL, TH = TLO[b], THI[b]
                    T = TL + TH
                    er = edgep.tile([128, t_max, 128], BF16, tag="er")
                    if TL:
                        nc.gpsimd.dma_gather(
                            er[:, :TL, :], table[l][0:half, :],
                            idx_lo[:, olo : olo + TL * 8],
                            TL * 128, TL * 128, 128,
                        )
                    if TH:
                        nc.gpsimd.dma_gather(
                            er[:, TL : TL + TH, :], table[l][half:n_nodes, :],
                            idx_hi[:, ohi : ohi + TH * 8],
                            TH * 128, TH * 128, 128,
                        )
                    msb = mp.tile([128, t_max, 128], BF16, tag="m")
                    nc.sync.dma_start(
                        msb[:, :T, :],
                        m_d[:, toff * 128 : (toff + T) * 128],
                    )
                    mtsb = mtp.tile([128, t_max, 128], BF16, tag="mt")
                    nc.scalar.dma_start(
                        mtsb[:, :T, :],
                        mt_d[:, toff * 128 : (toff + T) * 128],
                    )

                    # scores
                    prod = scp.tile([128, t_max, 128], BF16, tag="prod")
                    nc.vector.tensor_tensor(
                        out=prod[:, :T, :fo], in0=er[:, :T, :fo],
                        in1=ass[l][:].unsqueeze(1).broadcast_to((128, T, fo)),
                        op=ALU.mult,
                    )
                    a_s = scp.tile([128, t_max, hmax], F32, tag="a_s")
                    nc.vector.tensor_reduce(
                        out=a_s[:, :T, :H],
                        in_=prod[:, :T, :fo].rearrange(
                            "p t (h c) -> p t h c", h=H
                        ),
                        axis=mybir.AxisListType.X, op=ALU.add,
                    )
                    pad_ps = padp.tile([128, t_max, hmax], F32, tag="pad")
                    for j in range(T):
                        nc.tensor.matmul(
                            pad_ps[:, j, :H], lhsT=mtsb[:, j, :],
                            rhs=ad_store[:, b, :H], start=True, stop=True,
                        )
                    e_t = scp.tile([128, t_max, hmax], F32, tag="e_t")
                    nc.vector.tensor_tensor(
                        out=e_t[:, :T, :H], in0=a_s[:, :T, :H],
                        in1=pad_ps[:, :T, :H], op=ALU.add,
                    )
                    # leaky relu: max(e, 0.2e); then exp -> bf16 weights
                    lr = scp.tile([128, t_max, hmax], F32, tag="lr")
                    nc.vector.tensor_scalar(
                        out=lr[:, :T, :H], in0=e_t[:, :T, :H],
                        scalar1=NEG_SLOPE, scalar2=None, op0=ALU.mult,
                    )
                    nc.vector.tensor_tensor(
                        out=lr[:, :T, :H], in0=lr[:, :T, :H],
                        in1=e_t[:, :T, :H], op=ALU.max,
                    )
                    w_t = scp.tile([128, t_max, hmax], BF16, tag="w_t")
                    nc.scalar.activation(w_t[:, :T, :H], lr[:, :T, :H], AF.Exp)

                    g_t = gp.tile([128, t_max, fo + hmax], BF16, tag="g")
                    for hh in range(H):
                        nc.vector.tensor_tensor(
                            out=g_t[:, :T, hh * HID : (hh + 1) * HID],
                            in0=er[:, :T, hh * HID : (hh + 1) * HID],
                            in1=w_t[:, :T, hh].unsqueeze(-1).broadcast_to(
                                (128, T, HID)
                            ),
                            op=ALU.mult,
                        )
                    nc.scalar.copy(g_t[:, :T, fo : fo + H], w_t[:, :T, :H])

                    pblk = pblkp.tile([128, fo + H], F32, tag="blk")
                    for j in range(T):
                        nc.tensor.matmul(
                            pblk[:], lhsT=msb[:, j, :], rhs=g_t[:, j, : fo + H],
                            start=(j == 0), stop=(j == T - 1),
                        )

                    # epilogue: normalize + BN (+relu) + transpose/store
                    recip = scp.tile([128, hmax], F32, tag="rec")
                    nc.vector.reciprocal(recip[:, :H], pblk[:, fo : fo + H])
                    xr = scp.tile([128, 128], F32, tag="xr")
                    for hh in range(H):
                        nc.vector.tensor_scalar(
                            out=xr[:, hh * HID : (hh + 1) * HID],
                            in0=pblk[:, hh * HID : (hh + 1) * HID],
                            scalar1=recip[:, hh : hh + 1], scalar2=None,
                            op0=ALU.mult,
                        )
                    nc.vector.tensor_tensor(
                        out=xr[:, :fo], in0=xr[:, :fo], in1=scs[l][:],
                        op=ALU.mult,
                    )
                    nc.vector.tensor_tensor(
                        out=xr[:, :fo], in0=xr[:, :fo], in1=shs[l][:],
                        op=ALU.add,
                    )
                    if l < n_layers - 1:
                        nc.vector.tensor_scalar(
                            out=xr[:, :fo], in0=xr[:, :fo], scalar1=0.0,
                            scalar2=None, op0=ALU.max,
                        )
                        ptt = ptp.tile([128, 128], F32, tag="pt")
                        nc.tensor.transpose(ptt[:], xr[:], ident[:])
                        nc.scalar.copy(
                            xt_next[:, b * 128 : (b + 1) * 128], ptt[:]
                        )
                    else:
                        nrows = min(128, shard - b * 128)
                        nc.sync.dma_start(
                            out_d[b * 128 : b * 128 + nrows, :],
                            xr[:nrows, :fo],
                        )
                    olo += TL * 8
                    ohi += TH * 8
                    toff += T
    nc.compile()
    return nc


# ----------------------------------------------------------------------------
# public entry
# ----------------------------------------------------------------------------

def kernel(x, edge_index, params, _trace=False, _tmpdir=None):
    x = np.asarray(x, np.float32)
    edge_index = np.asarray(edge_index)
    n_nodes = x.shape[0]

    per_core, meta = prep_graph(edge_index, n_nodes, N_CORES)
    wmeta = prep_weights(params)
    nc = build_nc(meta, wmeta, n_nodes, N_CORES)

    shard, shard_pad = meta["shard"], meta["shard_pad"]
    fi0 = wmeta[0]["fi"]
    in_maps = []
    for k in range(N_CORES):
        xt0 = np.zeros((fi0, shard_pad), np.float32)
        xt0[:, :shard] = x[k * shard : (k + 1) * shard].T
        im = dict(
            xt0=xt0,
            idx_lo=per_core[k]["idx_lo"],
            idx_hi=per_core[k]["idx_hi"],
            m_tiles=per_core[k]["m_tiles"],
            mt_tiles=per_core[k]["mt_tiles"],
        )
        for l, w in enumerate(wmeta):
            im[f"wcat{l}"] = w["wcat"]
            im[f"asrc{l}"] = w["asrc"]
            im[f"scale{l}"] = w["scale"]
            im[f"shift{l}"] = w["shift"]
        in_maps.append(im)

    import time as _time

    t0 = _time.time()
    res = run_bass_kernel_spmd(
        nc, in_maps, list(range(N_CORES)), tmpdir=_tmpdir,
    )
    exec_wall_ns = (_time.time() - t0) * 1e9
    out = np.concatenate([np.asarray(r["out"]) for r in res.results], axis=0)
    if _trace:
        return out.astype(np.float32), exec_wall_ns
    return out.astype(np.float32)
